# revision 15
# baseline (speedup 1.0000x reference)
"""Trainium2 Bass kernel: 16-head MHA (B=2, T=2048, D=1024, d_k=64).

Sharding (8 NeuronCores): data-parallel over the batch (2) x tensor-parallel
over head groups (4 groups of 4 heads).  Core c handles batch b = c//4 and
heads [4g, 4g+4) with g = c%4.  Each core computes its partial output
    sum_{h in group} softmax((q Wq_h + bq_h)(k Wk_h)^T / 8) (v Wv_h) Wo_h
and the host sums the 4 partials per batch and adds the constant row
bo + bv @ Wo once.  bk is dropped: with the all-ones mask it shifts every
score row by a per-row constant, which softmax ignores exactly.

Numerics: every matmul operand is bf16 (fp32 PSUM accumulation everywhere);
running the PE in bf16 mode instead of fp32-HIGH is the main speed lever.
Softmax denominators accumulate in fp32; reciprocals via exp(-ln(x)) on the
scalar engine, batched over 8 partitions ([8, 512]) so the single-lane
serial cost is ~0.4us, then broadcast across partitions with rank-1 bf16
matmuls.  End-to-end error stays in the few-1e-3 relative class.

Per-core pipeline:
  1. Q^T, K^T, V^T projections from host-pretransposed X^T bf16 chunks
     streamed over HBM, contracted on the PE (bf16 in, fp32 PSUM out).
     Q^T/K^T live as two [128, 2048] bf16 head-pair tiles (head h on
     partitions (h%2)*64..); V^T is transposed back on the PE into 16
     [128, 260] bf16 "V_ext" tiles: per head 64 V columns plus a ones
     column that yields the softmax row sums for free in the attn*V matmul.
  2. Per (head pair, 1024-wide q slice, 128-wide k tile): scores transposed
     S'[k, q] = K Q^T into [128,1024] PSUM (two 512-free matmuls on
     disjoint PE row groups per head); exp on the scalar engine (1/8 scale
     folded in) emitting bf16; O^T accumulated over the 16 k tiles into
     [65, 1024] PSUM per head with the row-sum row.
  3. Row sums collect on partitions 0..7 of one [8, 512] tile; one batched
     Ln+Exp computes all reciprocals (bf16 out); per (head, half) a rank-1
     PE matmul broadcasts them across 64 partitions and the DVE multiplies
     the drained bf16 O^T against the PSUM broadcast into two
     head-pair-stacked [128, 2048] bf16 O^T tiles.
  4. Output projection: per 128-row tile, two C=128 bf16 matmuls against
     head-pair-stacked bf16 Wo tiles, fp32 drain, DMA out.
"""

import functools
import os

import ml_dtypes
import numpy as np

import concourse.bass as bass
import concourse.mybir as mybir
import concourse.tile as tile
from concourse import bacc
from concourse.bass_utils import run_bass_kernel_spmd
from concourse.masks import make_identity

F32 = mybir.dt.float32
BF16 = mybir.dt.bfloat16
AFT = mybir.ActivationFunctionType
BF = ml_dtypes.bfloat16

D = 1024          # model dim
T = 2048          # sequence length
B = 2             # batch
HEADS = 16        # total heads
DK = 64           # head dim
NCORES = 8
GH = 4            # heads per core
GD = GH * DK      # 256 projection cols per core
NF = D // 128     # 8 contraction chunks
NKT = T // 128    # 16 k/t tiles
SCALE = 1.0 / np.sqrt(np.float32(DK))  # 1/8

# Results of the last run (for test harness introspection: exec_time_ns etc.)
LAST_RESULTS = None


@functools.lru_cache(maxsize=1)
def _build_program():
    nc = bacc.Bacc("TRN2", target_bir_lowering=False, debug=False,
                   num_devices=NCORES)

    xqT = nc.declare_dram_parameter("xqT", [D, T], BF16, isOutput=False)
    xkT = nc.declare_dram_parameter("xkT", [D, T], BF16, isOutput=False)
    xvT = nc.declare_dram_parameter("xvT", [D, T], BF16, isOutput=False)
    wq = nc.declare_dram_parameter("wq", [128, NF * GD], BF16, isOutput=False)
    wk = nc.declare_dram_parameter("wk", [128, NF * GD], BF16, isOutput=False)
    wv = nc.declare_dram_parameter("wv", [128, NF * GD], BF16, isOutput=False)
    wo = nc.declare_dram_parameter("wo", [2, 128, D], BF16, isOutput=False)
    bqv = nc.declare_dram_parameter("bqv", [128, 2], F32, isOutput=False)
    out = nc.declare_dram_parameter("out", [T, D], BF16, isOutput=True)

    with tile.TileContext(nc) as tc:
        # ---- persistent pools -------------------------------------------
        with (
            tc.tile_pool(name="qk", bufs=4) as qk_pool,
            tc.tile_pool(name="vext", bufs=NKT) as vext_pool,
            tc.tile_pool(name="wop", bufs=2) as wo_pool,
            tc.tile_pool(name="otp", bufs=2) as ot_pool,
            tc.tile_pool(name="const", bufs=1) as const_pool,
        ):
            bqv_sb = const_pool.tile([128, 2], F32, tag="bqv")
            nc.sync.dma_start(bqv_sb[:], bqv[:])
            ident = const_pool.tile([128, 128], BF16, tag="ident")
            ones_f32 = const_pool.tile([128, DK], F32, tag="ones32")
            ones_sb = const_pool.tile([1, DK], BF16, tag="ones")

            QT = [qk_pool.tile([128, T], BF16, tag="qk", name=f"qt{m}")
                  for m in range(2)]
            KT = [qk_pool.tile([128, T], BF16, tag="qk", name=f"kt{m}")
                  for m in range(2)]
            VE = [vext_pool.tile([128, GH * (DK + 1)], BF16, tag="vext",
                                 name=f"ve{i}") for i in range(NKT)]
            WO = [wo_pool.tile([128, D], BF16, tag="wop", name=f"wo{m}")
                  for m in range(2)]
            OT = [ot_pool.tile([128, T], BF16, tag="ot", name=f"ot{m}")
                  for m in range(2)]

            # ---- phase A: projections -----------------------------------
            with (
                tc.tile_pool(name="wts", bufs=3) as w_pool,
                tc.tile_pool(name="xt", bufs=6) as xt_pool,
                tc.tile_pool(name="vt", bufs=2) as vt_pool,
                tc.tile_pool(name="psA", bufs=8,
                             space=bass.MemorySpace.PSUM) as psA,
            ):
                VT = [vt_pool.tile([128, T], BF16, tag="vt", name=f"vt{m}")
                      for m in range(2)]

                def projection(w_dram, x_dram, drain):
                    w_sb = w_pool.tile([128, NF * GD], BF16, tag="w")
                    nc.sync.dma_start(w_sb[:], w_dram[:])
                    ps = [psA.tile([128, 512], F32, tag="proj",
                                   name=f"pj{i}") for i in range(8)]
                    for fc in range(NF):
                        xt = xt_pool.tile([128, T], BF16, tag="xt")
                        nc.sync.dma_start(
                            xt[:], x_dram[fc * 128:(fc + 1) * 128, :])
                        for m in range(2):
                            for qh in range(4):
                                nc.tensor.matmul(
                                    ps[m * 4 + qh][:],
                                    w_sb[:, fc * GD + m * 128:
                                         fc * GD + (m + 1) * 128],
                                    xt[:, qh * 512:(qh + 1) * 512],
                                    start=(fc == 0), stop=(fc == NF - 1))
                    for m in range(2):
                        for qh in range(4):
                            drain(m, qh, ps[m * 4 + qh])

                def q_drain(m, qh, ps):
                    nc.vector.tensor_scalar_add(
                        QT[m][:, qh * 512:(qh + 1) * 512], ps[:],
                        bqv_sb[:, m:m + 1])

                def k_drain(m, qh, ps):
                    nc.vector.tensor_copy(
                        KT[m][:, qh * 512:(qh + 1) * 512], ps[:])

                def v_drain(m, qh, ps):
                    nc.vector.tensor_copy(
                        VT[m][:, qh * 512:(qh + 1) * 512], ps[:])

                projection(wq, xqT, q_drain)
                projection(wk, xkT, k_drain)
                projection(wv, xvT, v_drain)

                # constants for the transposes / broadcasts, prepared on
                # the gpsimd/vector engines while the PE projects
                make_identity(nc, ident[:])
                nc.gpsimd.memset(ones_f32[:], 1.0)
                nc.vector.tensor_copy(ones_sb[:], ones_f32[0:1, :])

                # V^T -> V_ext (PE transpose of 128x128 blocks, per pair)
                for kt in range(NKT):
                    ve = VE[kt]
                    ve_r = ve[:].rearrange("p (h x) -> p h x", x=DK + 1)
                    nc.vector.tensor_copy(
                        ve_r[:, :, DK:DK + 1],
                        ones_f32[:, 0:GH].rearrange("p (h x) -> p h x", x=1))
                    for m in range(2):
                        tp = psA.tile([128, 128], BF16, tag="proj")
                        nc.tensor.transpose(
                            tp[:], VT[m][:, kt * 128:(kt + 1) * 128],
                            ident[:])
                        nc.vector.tensor_copy(
                            ve_r[:, 2 * m:2 * m + 2, 0:DK],
                            tp[:].rearrange("k (h j) -> k h j", j=DK))

            nc.sync.dma_start(WO[0][:], wo[0])
            nc.sync.dma_start(WO[1][:], wo[1])

            # ---- phase B: attention + fused output projection -----------
            with (
                tc.tile_pool(name="ep", bufs=6) as epool,
                tc.tile_pool(name="ubp", bufs=8) as ub_pool,
                tc.tile_pool(name="rsp", bufs=2) as rs_pool,
                tc.tile_pool(name="osbp", bufs=4) as out_pool,
                tc.tile_pool(name="psS", bufs=2,
                             space=bass.MemorySpace.PSUM) as psS,
                tc.tile_pool(name="psO", bufs=1,
                             space=bass.MemorySpace.PSUM) as psO,
                tc.tile_pool(name="psR", bufs=2,
                             space=bass.MemorySpace.PSUM) as psR,
            ):
                # One flat stream of (qs, hp, hh, kt) steps.  Each step
                # emits its scores+exp, then the PREVIOUS step's attn*V
                # matmuls, so the in-order tensor queue always has a
                # full step of PE work queued ahead of every wait on the
                # scalar engine -- across pass and q-slice boundaries too.
                # The finish work of q slice 0 (reciprocal + normalize +
                # output projection) is drip-fed into the early steps of
                # q slice 1 on dedicated PSUM banks (psR), so only the
                # final q slice pays a serial tail.
                rs_t = {}
                rr_t = {}
                ub = {}
                o_cur = {"o": None, "h": None}
                prev = None          # (o_ps, h, kt, e, hp, hh)

                def drains(hp, hh, o_ps, qs):
                    for hf in range(2):
                        i8 = hp * 4 + hh * 2 + hf
                        u = ub_pool.tile([DK, 512], BF16, tag="ub",
                                         name=f"ub{qs}_{i8}")
                        nc.vector.tensor_copy(
                            u[:], o_ps[0:DK, hf * 512:(hf + 1) * 512])
                        p8 = 32 * (hh * 2 + hf)
                        nc.vector.tensor_copy(
                            rs_t[qs][p8:p8 + 1, hp * 512:(hp + 1) * 512],
                            o_ps[DK:DK + 1, hf * 512:(hf + 1) * 512])
                        ub[(qs, i8)] = u

                def recip(qs):
                    # batched 1/x via exp(-ln(x)) over 8 partitions at once
                    nc.scalar.activation(rs_t[qs][:], rs_t[qs][:], AFT.Ln)
                    nc.scalar.activation(rr_t[qs][:], rs_t[qs][:], AFT.Exp,
                                         scale=-1.0)

                def normalize(qs, i8):
                    q0 = qs * 1024
                    hp, hh, hf = i8 // 4, (i8 // 2) % 2, i8 % 2
                    m, lo = hp, hh * DK
                    p8 = 32 * (hh * 2 + hf)
                    # stage the reciprocal row onto partition 0 for the
                    # rank-1 broadcast matmul
                    rp = rs_pool.tile([1, 512], BF16, tag="rp",
                                      name=f"rp{qs}_{i8}")
                    nc.vector.tensor_copy(
                        rp[:], rr_t[qs][p8:p8 + 1, hp * 512:(hp + 1) * 512])
                    r_ps = psR.tile([DK, 512], F32, tag="rf",
                                    name=f"rps{qs}_{i8}")
                    nc.tensor.matmul(r_ps[:], ones_sb[:], rp[:],
                                     start=True, stop=True)
                    # odd heads land on partitions 64:128 of the
                    # head-pair-stacked O^T tile via the DVE write base
                    nc.vector.tensor_mul(
                        OT[m][lo:lo + DK, q0 + hf * 512:q0 + (hf + 1) * 512],
                        ub[(qs, i8)][:], r_ps[:])

                def outproj(tt):
                    osb = out_pool.tile([128, 1024], BF16, tag="osb")
                    for ei in range(2):
                        f_ps = psR.tile([128, 512], F32, tag="rf",
                                        name=f"fps{tt}_{ei}")
                        for m in range(2):
                            nc.tensor.matmul(
                                f_ps[:],
                                OT[m][:, tt * 128:(tt + 1) * 128],
                                WO[m][:, ei * 512:(ei + 1) * 512],
                                start=(m == 0), stop=(m == 1))
                        nc.vector.tensor_copy(
                            osb[:, ei * 512:(ei + 1) * 512], f_ps[:])
                    nc.sync.dma_start(out[tt * 128:(tt + 1) * 128, :],
                                      osb[:])

                def finish_work(qs, slot):
                    # slot 0.. within the next q slice's step stream
                    if slot == 0:
                        recip(qs)
                    elif 1 <= slot <= 4:
                        normalize(qs, 2 * (slot - 1))
                        normalize(qs, 2 * (slot - 1) + 1)
                    elif 5 <= slot <= 12:
                        outproj(qs * 8 + (slot - 5))

                def flush_prev():
                    po, ph, pkt, pe, php, phh, pqs = prev
                    for hf in range(2):
                        nc.tensor.matmul(
                            po[:, hf * 512:(hf + 1) * 512],
                            VE[pkt][:, ph * (DK + 1):(ph + 1) * (DK + 1)],
                            pe[:, hf * 512:(hf + 1) * 512],
                            start=(pkt == 0), stop=(pkt == NKT - 1))
                    if pkt == NKT - 1:
                        drains(php, phh, po, pqs)

                o_ps = None
                for qs in range(2):
                    rs_t[qs] = rs_pool.tile([128, 1024], F32, tag="rs",
                                            name=f"rs{qs}")
                    rr_t[qs] = rs_pool.tile([128, 1024], BF16, tag="rr",
                                            name=f"rr{qs}")
                    nc.gpsimd.memset(rs_t[qs][:], 1.0)
                    q0 = qs * 1024
                    for hp in range(2):      # head pairs -> PE row groups
                        for hh in range(2):  # one head per pass
                            h = hp * 2 + hh
                            lo = hh * DK
                            for kt in range(NKT):
                                ss = psS.tile([128, 1024], F32, tag="s")
                                for hf in range(2):
                                    nc.tensor.matmul(
                                        ss[:, hf * 512:(hf + 1) * 512],
                                        KT[hp][lo:lo + DK,
                                               kt * 128:(kt + 1) * 128],
                                        QT[hp][lo:lo + DK,
                                               q0 + hf * 512:
                                               q0 + (hf + 1) * 512],
                                        start=True, stop=True)
                                e = epool.tile([128, 1024], BF16, tag="e")
                                nc.scalar.activation(e[:], ss[:], AFT.Exp,
                                                     scale=float(SCALE))
                                if prev is not None:
                                    flush_prev()
                                if kt == 0:
                                    o_ps = psO.tile([65, 1024], F32,
                                                    tag="o",
                                                    name=f"o{qs}_{h}")
                                prev = (o_ps, h, kt, e, hp, hh, qs)
                                if qs == 1:
                                    step = (hp * 2 + hh) * NKT + kt
                                    if 2 <= step <= 14:
                                        finish_work(0, step - 2)

                # tail of the final q slice
                flush_prev()
                prev = None
                for slot in range(13):
                    finish_work(1, slot)

    from concourse.bacc import get_activation_tables
    import bass_rust as _br
    _combined = "natural_log_exp_and_others"
    _tabs = []
    for _name, _fns in get_activation_tables(nc.m.arch).items():
        if _name != _combined:
            _fns = _fns - {AFT.Exp, AFT.Ln}
        _tabs.append((_name, _fns))
    _br.insert_act_table_loads(nc, _tabs)
    nc.compile()
    return nc


def _numpy_reference(q, k, v, mask, Wq, bq, Wk, bk, Wv, bv, Wo, bo):
    """Fallback for a non-trivial mask (never hit with the stock inputs)."""
    Bn, Tn, _ = q.shape
    H, dk = HEADS, DK

    def split(x):
        return x.reshape(Bn, Tn, H, dk).transpose(0, 2, 1, 3)

    qh = split(q @ Wq + bq)
    kh = split(k @ Wk + bk)
    vh = split(v @ Wv + bv)
    s = np.einsum("bhqd,bhkd->bhqk", qh, kh) / np.sqrt(np.float32(dk))
    s = np.where(mask, s, -np.inf)
    s = s - s.max(axis=-1, keepdims=True)
    e = np.exp(s)
    a = e / e.sum(axis=-1, keepdims=True)
    o = np.einsum("bhqk,bhkd->bhqd", a, vh)
    o = o.transpose(0, 2, 1, 3).reshape(Bn, Tn, H * dk)
    return (o @ Wo + bo).astype(np.float32)


def kernel(q, k, v, mask, Wq, bq, Wk, bk, Wv, bv, Wo, bo):
    global LAST_RESULTS
    q = np.asarray(q, np.float32)
    k = np.asarray(k, np.float32)
    v = np.asarray(v, np.float32)
    mask = np.asarray(mask, bool)
    Wq, bq = np.asarray(Wq, np.float32), np.asarray(bq, np.float32)
    Wk, bk = np.asarray(Wk, np.float32), np.asarray(bk, np.float32)
    Wv, bv = np.asarray(Wv, np.float32), np.asarray(bv, np.float32)
    Wo, bo = np.asarray(Wo, np.float32), np.asarray(bo, np.float32)

    if not mask.all():
        return _numpy_reference(q, k, v, mask, Wq, bq, Wk, bk, Wv, bv, Wo, bo)

    nc = _build_program()

    # host-side sharding
    xT = {}
    for b in range(B):
        xT[b] = tuple(np.ascontiguousarray(x[b].T.astype(BF))
                      for x in (q, k, v))

    def w_chunks(W, g):
        # (1024, 256) head-group slice -> [128, 8*256] chunk-major layout
        Wg = W[:, g * GD:(g + 1) * GD]
        return np.ascontiguousarray(
            Wg.reshape(NF, 128, GD).transpose(1, 0, 2)
            .reshape(128, NF * GD).astype(BF))

    in_maps = []
    for c in range(NCORES):
        b, g = divmod(c, GH)
        xq_t, xk_t, xv_t = xT[b]
        in_maps.append({
            "xqT": xq_t, "xkT": xk_t, "xvT": xv_t,
            "wq": w_chunks(Wq, g), "wk": w_chunks(Wk, g),
            "wv": w_chunks(Wv, g),
            "wo": np.ascontiguousarray(
                Wo[g * GD:(g + 1) * GD, :].astype(BF)).reshape(2, 128, D),
            "bqv": np.ascontiguousarray(
                bq[g * GD:(g + 1) * GD].reshape(2, 128).T),
        })

    LAST_RESULTS = run_bass_kernel_spmd(
        nc, in_maps, list(range(NCORES)),
        trace=bool(os.environ.get("KERNEL_TRACE")))
    res = LAST_RESULTS.results

    const_row = (bv @ Wo + bo).astype(np.float32)  # attn rows sum to 1
    full = np.empty((B, T, D), np.float32)
    for b in range(B):
        acc = res[b * GH]["out"].astype(np.float32)
        for g in range(1, GH):
            acc = acc + res[b * GH + g]["out"]
        full[b] = acc + const_row
    return full
